# revision 7
# baseline (speedup 1.0000x reference)
"""Spatial-softmax expectation kernel for Trainium2, 8-core SPMD.

Computes, for x of shape [B=32, C=128, H=128, W=128]:
    prob = softmax(x.reshape(B, C, H*W), axis=-1)
    keypoints[b, c] = (sum_n prob[n] * xs[w(n)], sum_n prob[n] * ys[h(n)])

Strategy (per (b, c) row the result factorizes through marginals):
    e       = exp(x - 2)                    (softmax is shift-invariant)
    marg_h  = sum_w e                       -> num_y = sum_h ys * marg_h
    marg_w  = sum_h e                       -> num_x = sum_w xs * marg_w
    s       = sum marg_h
    k       = (num_x / s, num_y / s)

Mapping: rows (b, c) on SBUF partitions, H*W on the free dim, chunked.
  - DMA:     single sync-HWDGE queue, 1 MiB chunks — the roofline term.
  - ScalarE: exp (fp32 -> fp16; only e is quantized, sums stay fp32)
  - VectorE: marg_h via free-axis reduce over W per h-plane
  - TensorE: marg_w via identity-matmuls, 4 h-planes per instruction
             (rhs free=512 = one PSUM bank); the 4 plane-classes are
             folded (fused with the xs weighting) once per group.
  - Tail:    the last chunk of the last group is DMA'd as tapered
             slices ([1024,512,512]) into one tile so the final
             landing piece is small (short exposed critical path)
             without paying small-descriptor DMA cost on the bulk.
  - Output:  all 4 groups' results collect in one [128, 4, 2] tile and
             store with a single DMA at the end (y DRAM layout is
             [128, 8], unshuffled on the host) — mid-stream stores
             measurably disrupt the input DMA stream.
Sharding: data-parallel over B*C rows; core i takes rows [i*512, (i+1)*512).
"""

import numpy as np

import concourse.bacc as bacc
import concourse.mybir as mybir
import concourse.tile as tile
from concourse.bass_utils import run_bass_kernel_spmd

B, C, H, W = 32, 128, 128, 128
N_CORES = 8
ROWS = B * C                     # 4096 (b, c) rows total
ROWS_PER_CORE = ROWS // N_CORES  # 512
HW = H * W                       # 16384
GROUP = 128                      # rows per partition-group
CHUNK = 2048                     # free-dim elements per DMA/compute chunk
TAPER = (1024, 512, 512)         # DMA slice sizes for the final chunk
BUFS = 8                         # data-pool double-buffer depth
EXP_BIAS = -2.0                  # exp(x-2): fp16 range safety, cancels in ratio

FP32 = mybir.dt.float32
FP16 = mybir.dt.float16

_cache = {}


def build_program(rows=ROWS_PER_CORE, chunk=CHUNK, n_reps=1, loop_n=None,
                  bufs=BUFS, taper=TAPER):
    """Build and compile the single-core Bass program (run SPMD on 8 cores).

    n_reps > 1 statically repeats the computation in-program; loop_n wraps it
    in a hardware For_i loop (both for benchmarking: per-rep HW time is the
    slope of wall time vs rep count, launch overhead cancels).
    """
    n_groups = rows // GROUP
    n_chunks = HW // chunk
    assert sum(taper) == chunk

    nc = bacc.Bacc("TRN2", target_bir_lowering=False, debug=False)

    x_d = nc.dram_tensor("x", [rows, HW], FP32, kind="ExternalInput")
    xsb4_d = nc.dram_tensor("xsb4", [128, 512], FP32, kind="ExternalInput")
    ysb_d = nc.dram_tensor("ysb", [128, H], FP32, kind="ExternalInput")
    idf_d = nc.dram_tensor("idf", [128, 128], FP16, kind="ExternalInput")
    # y packed as [partition, group*2]: one contiguous 32B run per partition
    # (a [rows, 2] layout would need 512 8-byte DMA descriptors).
    y_d = nc.dram_tensor("y", [GROUP, 2 * n_groups], FP32, kind="ExternalOutput")

    with tile.TileContext(nc) as tc:
        with (
            tc.tile_pool(name="const", bufs=1) as cpool,
            tc.tile_pool(name="data", bufs=bufs) as dpool,
            tc.tile_pool(name="marg", bufs=2) as mpool,
            tc.tile_pool(name="small", bufs=2) as spool,
            tc.tile_pool(name="psum", bufs=2, space="PSUM") as ppool,
        ):
            xsb4 = cpool.tile([128, 512], FP32, tag="xsb4")
            ysb = cpool.tile([128, H], FP32, tag="ysb")
            idf = cpool.tile([128, 128], FP16, tag="idf")
            bias_t = cpool.tile([128, 1], FP32, tag="bias")
            nc.sync.dma_start(xsb4[:], xsb4_d[:, :])
            nc.sync.dma_start(ysb[:], ysb_d[:, :])
            nc.sync.dma_start(idf[:], idf_d[:, :])
            nc.vector.memset(bias_t[:], EXP_BIAS)

            def compute_piece(et, xt, margh, margw_ps, lo, size, off):
                # exp + marg_h + marg_w matmuls for xt[:, lo:lo+size] at
                # absolute element offset `off` within the group row.
                nc.scalar.activation(
                    et[:, lo : lo + size], xt[:, lo : lo + size],
                    mybir.ActivationFunctionType.Exp, bias=bias_t[:],
                )
                h_per = size // W
                h_base = off // W
                e3 = et[:, lo : lo + size].rearrange("p (h w) -> p h w", w=W)
                nc.vector.reduce_sum(
                    margh[:, h_base : h_base + h_per], e3,
                    axis=mybir.AxisListType.X,
                )
                for j in range(0, size, 512):
                    nc.tensor.matmul(
                        margw_ps[:], idf[:], et[:, lo + j : lo + j + 512],
                        start=(off + j == 0), stop=(off + j + 512 == HW),
                    )

            yall_holder = [None]

            def emit_group(g, last_group):
                rows_lo = g * GROUP
                margw_ps = ppool.tile([128, 512], FP32, tag="margw")
                margh = mpool.tile([128, H], FP32, tag="margh")

                for c in range(n_chunks):
                    off = c * chunk
                    xt = dpool.tile([128, chunk], FP32, tag="xt")
                    et = dpool.tile([128, chunk], FP16, tag="et")
                    if last_group and c == n_chunks - 1:
                        lo = 0
                        for size in taper:
                            nc.sync.dma_start(
                                xt[:, lo : lo + size],
                                x_d[rows_lo : rows_lo + GROUP,
                                    off + lo : off + lo + size],
                            )
                            compute_piece(et, xt, margh, margw_ps, lo, size, off + lo)
                            lo += size
                    else:
                        nc.sync.dma_start(
                            xt[:], x_d[rows_lo : rows_lo + GROUP, off : off + chunk]
                        )
                        compute_piece(et, xt, margh, margw_ps, 0, chunk, off)

                # tensor_tensor_reduce would fuse mul+reduce, but that opcode
                # hard-faults the exec unit on this runtime; use mul+reduce.
                scr_x = spool.tile([128, 512], FP32, tag="scrx")
                scr_y = spool.tile([128, H], FP32, tag="scry")
                num_xy = spool.tile([128, 2], FP32, tag="numxy")
                nc.vector.tensor_mul(scr_x[:], margw_ps[:], xsb4[:])
                nc.vector.reduce_sum(num_xy[:, 0:1], scr_x[:], axis=mybir.AxisListType.X)
                nc.vector.tensor_mul(scr_y[:], margh[:], ysb[:])
                nc.vector.reduce_sum(num_xy[:, 1:2], scr_y[:], axis=mybir.AxisListType.X)
                s = spool.tile([128, 1], FP32, tag="s")
                nc.vector.reduce_sum(s[:], margh[:], axis=mybir.AxisListType.X)
                recip = spool.tile([128, 1], FP32, tag="recip")
                nc.vector.reciprocal(recip[:], s[:])
                if yall_holder[0] is None:
                    yall = spool.tile([128, n_groups, 2], FP32, tag="yall")
                    yall_holder[0] = yall
                yall = yall_holder[0]
                nc.vector.tensor_scalar_mul(yall[:, g, :], num_xy[:], recip[:])
                if last_group:
                    nc.sync.dma_start(y_d[:, :], yall[:])
                    yall_holder[0] = None

            def emit_all():
                for _rep in range(n_reps):
                    for g in range(n_groups):
                        emit_group(g, last_group=(g == n_groups - 1))

            if loop_n is not None:
                with tc.For_i(0, loop_n, 1, hint_engines=(mybir.EngineType.PE,)):
                    emit_all()
            else:
                emit_all()

    nc.compile()
    return nc


def make_consts():
    xs = np.linspace(-1.0, 1.0, W).astype(np.float32)
    ys = np.linspace(-1.0, 1.0, H).astype(np.float32)
    return {
        "xsb4": np.ascontiguousarray(np.tile(xs, (128, 4))),
        "ysb": np.ascontiguousarray(np.tile(ys, (128, 1))),
        "idf": np.eye(128, dtype=np.float16),
    }


def unshard_y(y_all):
    """[N_CORES(*)128, 2*n_groups] packed output -> [B, C, 2]."""
    n_groups = ROWS_PER_CORE // GROUP
    y = np.asarray(y_all).reshape(N_CORES, GROUP, n_groups, 2)
    return np.ascontiguousarray(y.transpose(0, 2, 1, 3)).reshape(B, C, 2)


def kernel(x):
    x = np.ascontiguousarray(np.asarray(x), dtype=np.float32)
    assert x.shape == (B, C, H, W), x.shape

    if "nc" not in _cache:
        _cache["nc"] = build_program()
    nc = _cache["nc"]

    consts = make_consts()
    xf = x.reshape(N_CORES, ROWS_PER_CORE, HW)
    in_maps = [{"x": xf[i], **consts} for i in range(N_CORES)]
    res = run_bass_kernel_spmd(nc, in_maps, list(range(N_CORES))).results
    y = np.stack([res[i]["y"] for i in range(N_CORES)], axis=0)  # [8, 128, 8]
    return unshard_y(y)
